# revision 6
# baseline (speedup 1.0000x reference)
"""Trainium2 Bass kernel for capsule routing (nn_Capsule).

Reference computation:
    u_hat = einsum('bic,ce->bie', u_vecs, W).reshape(B, I, N, D).transpose(0,2,1,3)
    b = 0
    for r in range(3):
        c = softmax(b, axis=1)                      # over capsules n
        out = squash(einsum('bni,bnid->bnd', c, u_hat))
        if r < 2: b = einsum('bnd,bnid->bni', out, u_hat)
    return out    # (B, N, D)

Key algebraic restructuring (u_hat is never materialized; it is 32 MiB per
core and every use of it factors through u_vecs and W):
    round 0:  c uniform = 1/N  ->  out0 = squash((1/N) * (sum_i u[b,i,:]) @ W)
    logits[b,i,n] = sum_c u[b,i,c] * V[b,c,n],   V[b,c,n] = sum_d W[c,(n,d)] o[b,n,d]
    T[b,n,c]     = sum_i softmax(logits)[b,i,n] * u[b,i,c]
    pre[b,n,d]   = sum_c T[b,n,c] * W[c,(n,d)]   -> out = squash(pre)

Sharding: data-parallel over batch, 4 batches per core x 8 cores, W replicated.
"""

import numpy as np
from contextlib import ExitStack

import concourse.bass as bass
import concourse.bacc as bacc
import concourse.tile as tile
from concourse import mybir
from concourse.bass_utils import run_bass_kernel_spmd
from concourse.masks import make_identity

B, I, C = 32, 1024, 256
N, D = 32, 64
ND = N * D
ROUTINGS = 3
EPS = 1e-7
NCORES = 8
BL = B // NCORES  # batches per core
IC = I // 128     # i chunks of 128
CK = C // 128     # c chunks of 128
F32 = mybir.dt.float32
MULT = mybir.AluOpType.mult


def _capsule_body(ctx: ExitStack, tc: tile.TileContext, out_ap, u_ap, w_ap):
    nc = tc.nc

    const = ctx.enter_context(tc.tile_pool(name="const", bufs=1))
    persist = ctx.enter_context(tc.tile_pool(name="persist", bufs=1))
    work = ctx.enter_context(tc.tile_pool(name="work", bufs=2))

    # ---- constants ----
    ident = const.tile([128, 128], F32)
    make_identity(nc, ident[:])
    ones_col = const.tile([128, 1], F32)
    nc.vector.memset(ones_col[:], 1.0)
    ones_row = const.tile([1, 64], F32)
    nc.gpsimd.memset(ones_row[:], 1.0)
    eps_sb = const.tile([1, 1], F32)
    nc.gpsimd.memset(eps_sb[:], EPS)

    # ---- persistent SBUF tensors ----
    w_sb = persist.tile([128, CK, ND], F32)       # [q, ck, (n,d)]
    wt_sb = persist.tile([64, N, C], F32)         # [d, n, c]
    u_sb = persist.tile([128, BL, IC, C], F32)    # [p, b, ic, c]
    ut_sb = persist.tile([128, BL, CK, I], F32)   # [q, b, ck, i]
    st_sb = persist.tile([128, CK, BL], F32)      # [q, ck, b]  (column sums of u)

    # ---- load inputs ----
    for ck in range(CK):
        nc.sync.dma_start(out=w_sb[:, ck, :], in_=w_ap[ck * 128:(ck + 1) * 128, :])
    for b in range(BL):
        for ic in range(IC):
            nc.sync.dma_start(
                out=u_sb[:, b, ic, :],
                in_=u_ap[b, ic * 128:(ic + 1) * 128, :],
            )

    # ---- setup transposes (PE) ----
    with tc.tile_pool(name="ps_setup", bufs=2, space="PSUM") as ps_setup:
        # W blocks:  wt[d, n, ck*128:+128] = W[ck-chunk, n-block].T
        for ck in range(CK):
            for n in range(N):
                wt_ps = ps_setup.tile([64, 128], F32, tag="wt")
                nc.tensor.transpose(
                    wt_ps[:], w_sb[:, ck, n * 64:(n + 1) * 64], ident[:]
                )
                if n % 2 == 0:
                    nc.vector.tensor_copy(
                        out=wt_sb[0:64, n, ck * 128:(ck + 1) * 128], in_=wt_ps[:]
                    )
                else:
                    nc.scalar.copy(
                        out=wt_sb[0:64, n, ck * 128:(ck + 1) * 128], in_=wt_ps[:]
                    )
        # u blocks: ut[q, b, ck, ic*128:+128] = u[b, i-chunk, c-chunk].T
        for b in range(BL):
            for ck in range(CK):
                for ic in range(IC):
                    ut_ps = ps_setup.tile([128, 128], F32, tag="ut")
                    nc.tensor.transpose(
                        ut_ps[:], u_sb[:, b, ic, ck * 128:(ck + 1) * 128], ident[:]
                    )
                    if (ic + ck) % 2 == 0:
                        nc.vector.tensor_copy(
                            out=ut_sb[:, b, ck, ic * 128:(ic + 1) * 128], in_=ut_ps[:]
                        )
                    else:
                        nc.scalar.copy(
                            out=ut_sb[:, b, ck, ic * 128:(ic + 1) * 128], in_=ut_ps[:]
                        )
        # column sums of u: st[q, ck, b] = sum_i u[b, i, ck-chunk]
        for b in range(BL):
            for ck in range(CK):
                nc.vector.reduce_sum(
                    out=st_sb[:, ck, b:b + 1].rearrange("q one -> q one"),
                    in_=ut_sb[:, b, ck, :],
                    axis=mybir.AxisListType.X,
                )

    ps = ctx.enter_context(tc.tile_pool(name="ps_main", bufs=1, space="PSUM"))
    ps2 = ctx.enter_context(tc.tile_pool(name="ps_lg", bufs=2, space="PSUM"))

    o_sb = None
    for r in range(ROUTINGS):
        # ---------- V (rounds >= 1): V[b][c, n] = sum_d W[c,(n,d)] o[b,n,d] ----------
        if r > 0:
            v_ps = ps.tile([128, CK, N, BL], F32, tag="v")
            for n in range(N):
                for ck in range(CK):
                    nc.tensor.matmul(
                        out=v_ps[:, ck, n, :],
                        lhsT=wt_sb[0:64, n, ck * 128:(ck + 1) * 128],
                        rhs=o_sb[:, n, :],
                        start=True,
                        stop=True,
                    )
            v_sb = work.tile([128, CK, N, BL], F32, tag="v_sb")
            nc.scalar.copy(out=v_sb[:], in_=v_ps[:])

            # ---------- logits + softmax + T + T^T, per local batch ----------
            tt_ps = ps.tile([128, CK, BL, N], F32, tag="tt")
            tt_sb = work.tile([128, CK, BL, N], F32, tag="tt_sb")
            for b in range(BL):
                lg_ps = ps2.tile([128, IC, N], F32, tag="lg")
                for ic in range(IC):
                    for ck in range(CK):
                        nc.tensor.matmul(
                            out=lg_ps[:, ic, :],
                            lhsT=ut_sb[:, b, ck, ic * 128:(ic + 1) * 128],
                            rhs=v_sb[:, ck, :, b],
                            start=(ck == 0),
                            stop=(ck == CK - 1),
                        )
                # softmax over n (free dim), no max-subtraction needed:
                # logits are O(1) so exp is safely in range.
                e_sb = work.tile([128, IC, N], F32, tag="e")
                nc.scalar.activation(
                    out=e_sb[:], in_=lg_ps[:], func=mybir.ActivationFunctionType.Exp
                )
                s_sb = work.tile([128, IC], F32, tag="s")
                nc.vector.reduce_sum(
                    out=s_sb[:], in_=e_sb[:], axis=mybir.AxisListType.X
                )
                sr_sb = work.tile([128, IC], F32, tag="sr")
                nc.vector.reciprocal(out=sr_sb[:], in_=s_sb[:])
                c_sb = work.tile([128, IC, N], F32, tag="c")
                nc.vector.tensor_tensor(
                    c_sb[:],
                    e_sb[:],
                    sr_sb[:, :, None].to_broadcast([128, IC, N]),
                    MULT,
                )
                # T[b][n, c] = sum_i c[i, n] u[b, i, c]
                t_ps = ps.tile([32, C], F32, tag="t")
                for ic in range(IC):
                    nc.tensor.matmul(
                        out=t_ps[:],
                        lhsT=c_sb[:, ic, :],
                        rhs=u_sb[:, b, ic, :],
                        start=(ic == 0),
                        stop=(ic == IC - 1),
                    )
                t_sb = work.tile([32, C], F32, tag="t_sb")
                nc.scalar.copy(out=t_sb[:], in_=t_ps[:])
                for ck in range(CK):
                    nc.tensor.transpose(
                        tt_ps[:, ck, b, :],
                        t_sb[:, ck * 128:(ck + 1) * 128],
                        ident[0:32, 0:32],
                    )
            nc.vector.tensor_copy(out=tt_sb[:], in_=tt_ps[:])

        # ---------- pre[d, n, b] ----------
        pre_ps = ps.tile([64, N, BL], F32, tag="pre")
        for n in range(N):
            for ck in range(CK):
                rhs = (
                    st_sb[:, ck, :] if r == 0 else tt_sb[:, ck, :, n]
                )
                nc.tensor.matmul(
                    out=pre_ps[:, n, :],
                    lhsT=w_sb[:, ck, n * 64:(n + 1) * 64],
                    rhs=rhs,
                    start=(ck == 0),
                    stop=(ck == CK - 1),
                )

        # ---------- squash ----------
        pre_sb = work.tile([64, N, BL], F32, tag="pre_sb")
        if r == 0:
            nc.scalar.mul(out=pre_sb[:], in_=pre_ps[:], mul=1.0 / N)
        else:
            nc.scalar.copy(out=pre_sb[:], in_=pre_ps[:])
        sq_sb = work.tile([64, N * BL], F32, tag="sq")
        pre2d = pre_sb[:].rearrange("d n b -> d (n b)")
        nc.vector.tensor_mul(sq_sb[:], pre2d, pre2d)
        ssum_ps = ps.tile([1, N * BL], F32, tag="ssum")
        nc.tensor.matmul(
            out=ssum_ps[:], lhsT=ones_col[0:64, :], rhs=sq_sb[:],
            start=True, stop=True,
        )
        rt_sb = work.tile([1, N * BL], F32, tag="rt")
        nc.scalar.activation(
            out=rt_sb[:], in_=ssum_ps[:],
            func=mybir.ActivationFunctionType.Sqrt, bias=eps_sb[:],
        )
        rn_sb = work.tile([1, N * BL], F32, tag="rn")
        nc.vector.reciprocal(out=rn_sb[:], in_=rt_sb[:])
        rnb_ps = ps.tile([64, N * BL], F32, tag="rnb")
        nc.tensor.matmul(
            out=rnb_ps[:], lhsT=ones_row[:], rhs=rn_sb[:], start=True, stop=True
        )
        o_sb = work.tile([64, N, BL], F32, tag="o")
        nc.vector.tensor_tensor(
            o_sb[:].rearrange("d n b -> d (n b)"), pre2d, rnb_ps[:], MULT
        )

    # ---------- write out: out[b, n, d] = o[d, n, b] ----------
    ot_ps = ps.tile([128, 64], F32, tag="rnb")  # reuse bank (rnb is dead)
    nc.tensor.transpose(
        ot_ps[:], o_sb[:].rearrange("d n b -> d (n b)"), ident[0:64, 0:64]
    )
    ot_sb = work.tile([128, 64], F32, tag="ot")
    nc.scalar.copy(out=ot_sb[:], in_=ot_ps[:])
    # dest iterated as (n, b, d) to match ot_sb's (n*BL+b) partition order
    out_nbd = bass.AP(
        tensor=out_ap.tensor,
        offset=out_ap.offset,
        ap=[[D, N], [N * D, BL], [1, D]],
    )
    nc.sync.dma_start(out=out_nbd, in_=ot_sb[:])


def build_program():
    nc = bacc.Bacc("TRN2", target_bir_lowering=False, debug=False)
    u_ap = nc.dram_tensor("u", [BL, I, C], F32, kind="ExternalInput").ap()
    w_ap = nc.dram_tensor("w", [C, ND], F32, kind="ExternalInput").ap()
    out_ap = nc.dram_tensor("out", [BL, N, D], F32, kind="ExternalOutput").ap()
    with tile.TileContext(nc) as tc:
        with ExitStack() as ctx:
            _capsule_body(ctx, tc, out_ap, u_ap, w_ap)
    nc.compile()
    return nc


_NC = None


def kernel(u_vecs: np.ndarray, W: np.ndarray) -> np.ndarray:
    global _NC
    u = np.ascontiguousarray(np.asarray(u_vecs, dtype=np.float32))
    w = np.ascontiguousarray(np.asarray(W, dtype=np.float32))
    assert u.shape == (B, I, C) and w.shape == (C, ND)
    if _NC is None:
        _NC = build_program()
    in_maps = [
        {"u": u[i * BL:(i + 1) * BL], "w": w} for i in range(NCORES)
    ]
    res = run_bass_kernel_spmd(_NC, in_maps, list(range(NCORES)))
    return np.concatenate(
        [res.results[i]["out"] for i in range(NCORES)], axis=0
    )


# revision 10
# speedup vs baseline: 1.0041x; 1.0041x over previous
"""Trainium2 Bass kernel for capsule routing (nn_Capsule).

Reference computation:
    u_hat = einsum('bic,ce->bie', u_vecs, W).reshape(B, I, N, D).transpose(0,2,1,3)
    b = 0
    for r in range(3):
        c = softmax(b, axis=1)                      # over capsules n
        out = squash(einsum('bni,bnid->bnd', c, u_hat))
        if r < 2: b = einsum('bnd,bnid->bni', out, u_hat)
    return out    # (B, N, D)

Key algebraic restructuring (u_hat is never materialized; it is 32 MiB per
core and every use of it factors through u_vecs and W):
    round 0:  c uniform = 1/N  ->  out0 = squash((1/N) * (sum_i u[b,i,:]) @ W)
    logits[b,i,n] = sum_c u[b,i,c] * V[b,c,n],   V[b,c,n] = sum_d W[c,(n,d)] o[b,n,d]
    T[b,n,c]     = sum_i softmax(logits)[b,i,n] * u[b,i,c]
    pre[b,n,d]   = sum_c T[b,n,c] * W[c,(n,d)]   -> out = squash(pre)

Sharding: data-parallel over batch, 4 batches per core x 8 cores, W replicated.
"""

import numpy as np
from contextlib import ExitStack

import concourse.bass as bass
import concourse.bacc as bacc
import concourse.tile as tile
from concourse import mybir
from concourse.bass_utils import run_bass_kernel_spmd
from concourse.masks import make_identity

B, I, C = 32, 1024, 256
N, D = 32, 64
ND = N * D
ROUTINGS = 3
EPS = 1e-7
NCORES = 8
BL = B // NCORES  # batches per core
IC = I // 128     # i chunks of 128
CK = C // 128     # c chunks of 128
F32 = mybir.dt.float32
MULT = mybir.AluOpType.mult


def _capsule_body(ctx: ExitStack, tc: tile.TileContext, out_ap, u_ap, w_ap):
    nc = tc.nc

    const = ctx.enter_context(tc.tile_pool(name="const", bufs=1))
    persist = ctx.enter_context(tc.tile_pool(name="persist", bufs=1))
    work = ctx.enter_context(tc.tile_pool(name="work", bufs=2))

    # ---- constants ----
    ident = const.tile([128, 128], F32)
    make_identity(nc, ident[:])
    ones_col = const.tile([128, 1], F32)
    nc.vector.memset(ones_col[:], 1.0)
    ones_row = const.tile([1, 64], F32)
    nc.gpsimd.memset(ones_row[:], 1.0)
    eps_sb = const.tile([1, 1], F32)
    nc.gpsimd.memset(eps_sb[:], EPS)

    # ---- persistent SBUF tensors ----
    w_sb = persist.tile([128, CK, ND], F32)       # [q, ck, (n,d)]
    wt_sb = persist.tile([64, N, C], F32)         # [d, n, c]
    u_sb = persist.tile([128, BL, IC, C], F32)    # [p, b, ic, c]
    ut_sb = persist.tile([128, BL, CK, I], F32)   # [q, b, ck, i]
    st_sb = persist.tile([128, CK, BL], F32)      # [q, ck, b]  (column sums of u)

    # ---- load inputs ----
    for ck in range(CK):
        nc.sync.dma_start(out=w_sb[:, ck, :], in_=w_ap[ck * 128:(ck + 1) * 128, :])
    for b in range(BL):
        for ic in range(IC):
            nc.sync.dma_start(
                out=u_sb[:, b, ic, :],
                in_=u_ap[b, ic * 128:(ic + 1) * 128, :],
            )

    # ---- setup transposes (PE) ----
    with tc.tile_pool(name="ps_setup", bufs=2, space="PSUM") as ps_setup, \
            nc.named_scope("setup"):
        # W blocks:  wt[d, n, ck*128:+128] = W[ck-chunk, n-block].T
        for ck in range(CK):
            for n in range(N):
                wt_ps = ps_setup.tile([64, 128], F32, tag="wt")
                nc.tensor.transpose(
                    wt_ps[:], w_sb[:, ck, n * 64:(n + 1) * 64], ident[:]
                )
                if n % 2 == 0:
                    nc.vector.tensor_copy(
                        out=wt_sb[0:64, n, ck * 128:(ck + 1) * 128], in_=wt_ps[:]
                    )
                else:
                    nc.scalar.copy(
                        out=wt_sb[0:64, n, ck * 128:(ck + 1) * 128], in_=wt_ps[:]
                    )
        # u blocks: ut[q, b, ck, ic*128:+128] = u[b, i-chunk, c-chunk].T
        for b in range(BL):
            for ck in range(CK):
                for ic in range(IC):
                    ut_ps = ps_setup.tile([128, 128], F32, tag="ut")
                    nc.tensor.transpose(
                        ut_ps[:], u_sb[:, b, ic, ck * 128:(ck + 1) * 128], ident[:]
                    )
                    if (ic + ck) % 2 == 0:
                        nc.vector.tensor_copy(
                            out=ut_sb[:, b, ck, ic * 128:(ic + 1) * 128], in_=ut_ps[:]
                        )
                    else:
                        nc.scalar.copy(
                            out=ut_sb[:, b, ck, ic * 128:(ic + 1) * 128], in_=ut_ps[:]
                        )
        # column sums of u: st[q, ck, b] = sum_i u[b, i, ck-chunk]
        for b in range(BL):
            for ck in range(CK):
                nc.vector.reduce_sum(
                    out=st_sb[:, ck, b:b + 1].rearrange("q one -> q one"),
                    in_=ut_sb[:, b, ck, :],
                    axis=mybir.AxisListType.X,
                )

    ps = ctx.enter_context(tc.tile_pool(name="ps_main", bufs=1, space="PSUM"))
    ps2 = ctx.enter_context(tc.tile_pool(name="ps_lg", bufs=2, space="PSUM"))

    o_sb = None
    for r in range(ROUTINGS):
        # ---------- V (rounds >= 1): V[b][c, n] = sum_d W[c,(n,d)] o[b,n,d] ----------
        if r > 0:
            with nc.named_scope(f"r{r}_v"):
                v_ps = ps.tile([128, CK, N, BL], F32, tag="v")
                for n in range(N):
                    for ck in range(CK):
                        nc.tensor.matmul(
                            out=v_ps[:, ck, n, :],
                            lhsT=wt_sb[0:64, n, ck * 128:(ck + 1) * 128],
                            rhs=o_sb[:, n, :],
                            start=True,
                            stop=True,
                        )
                v_sb = work.tile([128, CK, N, BL], F32, tag="v_sb")
                nc.scalar.copy(out=v_sb[:], in_=v_ps[:])

            # ---------- logits + softmax + T + T^T, per local batch ----------
            tt_ps = ps.tile([128, CK, BL, N], F32, tag="tt")
            tt_sb = work.tile([128, CK, BL, N], F32, tag="tt_sb")
            for b in range(BL):
                lg_ps = ps2.tile([128, IC, N], F32, tag="lg")
                with nc.named_scope(f"r{r}_lg"):
                    for ic in range(IC):
                        for ck in range(CK):
                            nc.tensor.matmul(
                                out=lg_ps[:, ic, :],
                                lhsT=ut_sb[:, b, ck, ic * 128:(ic + 1) * 128],
                                rhs=v_sb[:, ck, :, b],
                                start=(ck == 0),
                                stop=(ck == CK - 1),
                            )
                # softmax over n (free dim), no max-subtraction needed:
                # logits are O(1) so exp is safely in range.
                with nc.named_scope(f"r{r}_sm"):
                    e_sb = work.tile([128, IC, N], F32, tag="e")
                    nc.scalar.activation(
                        out=e_sb[:], in_=lg_ps[:],
                        func=mybir.ActivationFunctionType.Exp,
                    )
                    s_sb = work.tile([128, IC], F32, tag="s")
                    nc.vector.reduce_sum(
                        out=s_sb[:], in_=e_sb[:], axis=mybir.AxisListType.X
                    )
                    sr_sb = work.tile([128, IC], F32, tag="sr")
                    nc.vector.reciprocal(out=sr_sb[:], in_=s_sb[:])
                    c_sb = work.tile([128, IC, N], F32, tag="c")
                    nc.vector.tensor_tensor(
                        c_sb[:],
                        e_sb[:],
                        sr_sb[:, :, None].to_broadcast([128, IC, N]),
                        MULT,
                    )
                # T[b][n, c] = sum_i c[i, n] u[b, i, c]
                with nc.named_scope(f"r{r}_t"):
                    t_ps = ps.tile([32, C], F32, tag="t")
                    for ic in range(IC):
                        nc.tensor.matmul(
                            out=t_ps[:],
                            lhsT=c_sb[:, ic, :],
                            rhs=u_sb[:, b, ic, :],
                            start=(ic == 0),
                            stop=(ic == IC - 1),
                        )
                    t_sb = work.tile([32, C], F32, tag="t_sb")
                    nc.scalar.copy(out=t_sb[:], in_=t_ps[:])
                    for ck in range(CK):
                        nc.tensor.transpose(
                            tt_ps[:, ck, b, :],
                            t_sb[:, ck * 128:(ck + 1) * 128],
                            ident[0:32, 0:32],
                        )
            nc.vector.tensor_copy(out=tt_sb[:], in_=tt_ps[:])

        # ---------- pre[d, n, b] ----------
        with nc.named_scope(f"r{r}_pre"):
            pre_ps = ps.tile([64, N, BL], F32, tag="pre")
            for n in range(N):
                for ck in range(CK):
                    rhs = (
                        st_sb[:, ck, :] if r == 0 else tt_sb[:, ck, :, n]
                    )
                    nc.tensor.matmul(
                        out=pre_ps[:, n, :],
                        lhsT=w_sb[:, ck, n * 64:(n + 1) * 64],
                        rhs=rhs,
                        start=(ck == 0),
                        stop=(ck == CK - 1),
                    )

        # ---------- squash ----------
        pre_sb = work.tile([64, N, BL], F32, tag="pre_sb")
        if r == 0:
            nc.scalar.mul(out=pre_sb[:], in_=pre_ps[:], mul=1.0 / N)
        else:
            nc.scalar.copy(out=pre_sb[:], in_=pre_ps[:])
        sq_sb = work.tile([64, N * BL], F32, tag="sq")
        pre2d = pre_sb[:].rearrange("d n b -> d (n b)")
        nc.vector.tensor_mul(sq_sb[:], pre2d, pre2d)
        ssum_ps = ps.tile([1, N * BL], F32, tag="ssum")
        nc.tensor.matmul(
            out=ssum_ps[:], lhsT=ones_col[0:64, :], rhs=sq_sb[:],
            start=True, stop=True,
        )
        rt_sb = work.tile([1, N * BL], F32, tag="rt")
        nc.scalar.activation(
            out=rt_sb[:], in_=ssum_ps[:],
            func=mybir.ActivationFunctionType.Sqrt, bias=eps_sb[:],
        )
        rn_sb = work.tile([1, N * BL], F32, tag="rn")
        nc.vector.reciprocal(out=rn_sb[:], in_=rt_sb[:])
        rnb_ps = ps.tile([64, N * BL], F32, tag="rnb")
        nc.tensor.matmul(
            out=rnb_ps[:], lhsT=ones_row[:], rhs=rn_sb[:], start=True, stop=True
        )
        o_sb = work.tile([64, N, BL], F32, tag="o")
        nc.vector.tensor_tensor(
            o_sb[:].rearrange("d n b -> d (n b)"), pre2d, rnb_ps[:], MULT
        )

    # ---------- write out: out[b, n, d] = o[d, n, b] ----------
    ot_ps = ps.tile([128, 64], F32, tag="rnb")  # reuse bank (rnb is dead)
    nc.tensor.transpose(
        ot_ps[:], o_sb[:].rearrange("d n b -> d (n b)"), ident[0:64, 0:64]
    )
    ot_sb = work.tile([128, 64], F32, tag="ot")
    nc.scalar.copy(out=ot_sb[:], in_=ot_ps[:])
    # dest iterated as (n, b, d) to match ot_sb's (n*BL+b) partition order
    out_nbd = bass.AP(
        tensor=out_ap.tensor,
        offset=out_ap.offset,
        ap=[[D, N], [N * D, BL], [1, D]],
    )
    nc.sync.dma_start(out=out_nbd, in_=ot_sb[:])


def build_program():
    nc = bacc.Bacc("TRN2", target_bir_lowering=False, debug=False)
    u_ap = nc.dram_tensor("u", [BL, I, C], F32, kind="ExternalInput").ap()
    w_ap = nc.dram_tensor("w", [C, ND], F32, kind="ExternalInput").ap()
    out_ap = nc.dram_tensor("out", [BL, N, D], F32, kind="ExternalOutput").ap()
    with tile.TileContext(nc) as tc:
        with ExitStack() as ctx:
            _capsule_body(ctx, tc, out_ap, u_ap, w_ap)
    nc.compile()
    return nc


_NC = None


def kernel(u_vecs: np.ndarray, W: np.ndarray) -> np.ndarray:
    global _NC
    u = np.ascontiguousarray(np.asarray(u_vecs, dtype=np.float32))
    w = np.ascontiguousarray(np.asarray(W, dtype=np.float32))
    assert u.shape == (B, I, C) and w.shape == (C, ND)
    if _NC is None:
        _NC = build_program()
    in_maps = [
        {"u": u[i * BL:(i + 1) * BL], "w": w} for i in range(NCORES)
    ]
    res = run_bass_kernel_spmd(_NC, in_maps, list(range(NCORES)))
    return np.concatenate(
        [res.results[i]["out"] for i in range(NCORES)], axis=0
    )
